# revision 1
# baseline (speedup 1.0000x reference)
"""Multivariate Hawkes log-likelihood on 8 Trainium2 NeuronCores.

Math: the pairwise kernel exp(-beta*(t_i - t_j)) is factorized per index-block
of 128 events: with tref_b = times[b*128],
    exp(-(t_i - t_j)) = exp(-(t_i - tref_B)) * exp(-(tref_B - tref_b)) * exp(t_j - tref_b)
so the O(N^2) history sum collapses to per-block accumulators
    W[b, s'] = sum_{j in b} exp(t_j - tref_b) * [sp_j == s']      (PE matmul)
    A[b, s'] = sum_{b' < b} exp(-(tref_b - tref_b')) * W[b', s']  (PE matmul, masked decay matrix)
plus an exact in-block strict-lower-triangular matmul. term2 (the integral) is
a dense masked [128, 32*GS] exp + 32 accumulating matmuls.

Sharding: row blocks {c, 31-c, 8+c, 23-c} per core (balanced causal work),
time-grid columns [c*13, c*13+13) per core. Per-core behavior differs only in
DATA (host-gathered slices / one-hot selectors), so one SPMD program serves
all 8 cores. Host does O(N) layout prep + the final 16-scalar gather-sum.

The walrus build in this container rejects any instruction with more than one
embedded sync wait, so: all inputs ride in ONE packed [128, X] blob (single DMA
-> single DMA semaphore), each engine "touches" the blob once to absorb that
semaphore into its vector clock, and the kernel-tail drain is split into one
Drain per outstanding semaphore (monkeypatch below).
"""

import numpy as np

import concourse.bass as bass
import concourse.mybir as mybir
import concourse.tile as tile
from concourse.bass_utils import run_bass_kernel_spmd
from concourse.vector_clock import ScopedClock, VectorClock


def _split_drain_and_barrier(self, tick_clock, wait_clock):
    gclock = tick_clock.global_clock
    for proc in range(len(gclock)):
        tick = gclock[proc]
        if tick <= 0:
            continue
        vc1 = VectorClock()
        vc1.require_at_least(proc, tick)
        di = self.nc.sync.drain()
        wait_clock.add_sem_waits(di.ins, ScopedClock({None: vc1}))
    self.nc.all_engine_barrier()
    assert self.sems is not None
    popped = self.nc._tile_sem_poison_stack.pop()
    assert popped is self._sem_poison
    self.nc.clear_and_free_semaphores(list(self.sems.allocated().values()))
    self.nc.all_engine_barrier()


tile.TileContext._drain_and_barrier = _split_drain_and_barrier

N, NB, BS, S, G, GS = 4096, 32, 128, 64, 100, 13
T0, T1, INT_RES = 0.0, 100.0, 100
NBLK = 4          # row blocks per core
W2 = NB * GS      # term2 batched free dim = 416
BIG = 1.0e5
F32 = mybir.dt.float32
AF = mybir.ActivationFunctionType
ALU = mybir.AluOpType

# packed input blob layout: name -> (rows, cols); offsets assigned in order
_FIELDS = [
    ("tdel_all", BS, NB), ("tdel_my", BS, NBLK), ("trefd_m", NB, NB),
    ("sel", NB, NBLK), ("onehot_t", BS, NB * S), ("onehot_my", BS, NBLK * S),
    ("alphag_my", BS, NBLK * S), ("mug_my", BS, NBLK), ("lmask", BS, BS),
    ("tg_t2", BS, W2), ("times_t2", BS, W2), ("alpha_t", S, S),
    ("mu_col", S, 1), ("ident64", S, S), ("gmask", S, GS),
]
_OFF = {}
_cur = 0
for _nm, _r, _c in _FIELDS:
    _OFF[_nm] = _cur
    _cur += _c
BLOB_COLS = _cur

_CACHE = {}


def _build_program():
    nc = bass.Bass()
    blob = nc.dram_tensor("blob", [BS, BLOB_COLS], F32, kind="ExternalInput")
    outd = nc.dram_tensor("outd", [1, 2], F32, kind="ExternalOutput")

    with tile.TileContext(nc) as tc:
        with (
            tc.tile_pool(name="const", bufs=1) as cp,
            tc.tile_pool(name="work", bufs=4) as wp,
            tc.tile_pool(name="ps1", bufs=1, space=bass.MemorySpace.PSUM) as ps1,
            tc.tile_pool(name="ps2", bufs=1, space=bass.MemorySpace.PSUM) as ps2,
        ):
            bsb = cp.tile([BS, BLOB_COLS], F32, tag="bsb")
            nc.sync.dma_start(bsb[:], blob[:])

            def fld(name, rows=None):
                r = dict((n, (rr, cc)) for n, rr, cc in _FIELDS)[name]
                off = _OFF[name]
                return bsb[0:(rows or r[0]), off:off + r[1]]

            # one touch per engine: absorb the blob-DMA semaphore into each
            # engine's vector clock so no later instruction needs a DMA wait.
            ptch = ps1.tile([1, 1], F32, tag="ptch")
            nc.tensor.matmul(ptch[:], bsb[0:1, 0:1], bsb[0:1, 0:1],
                             start=True, stop=True)
            dvetch = cp.tile([1, 1], F32, tag="dvetch")
            nc.vector.tensor_copy(dvetch[:], bsb[0:1, 0:1])
            acttch = cp.tile([1, 1], F32, tag="acttch")
            nc.scalar.copy(acttch[:], bsb[0:1, 0:1])
            sptch = cp.tile([1, 1], F32, tag="sptch")
            nc.sync.dma_start(sptch[:], bsb[0:1, 0:1])

            ones_row = cp.tile([1, BS], F32, tag="ones_row")
            nc.vector.memset(ones_row[:], 1.0)
            ones_col = cp.tile([BS, 1], F32, tag="ones_col")
            nc.vector.memset(ones_col[:], 1.0)

            # ---- preamble: v, u, D ----
            v_t = cp.tile([BS, NB], F32, tag="v_t")
            nc.scalar.activation(v_t[:], fld("tdel_all"), AF.Exp)
            vmy_t = cp.tile([BS, NBLK], F32, tag="vmy_t")
            nc.scalar.activation(vmy_t[:], fld("tdel_my"), AF.Exp)
            umy_t = cp.tile([BS, NBLK], F32, tag="umy_t")
            nc.scalar.activation(umy_t[:], fld("tdel_my"), AF.Exp, scale=-1.0)
            d_t = cp.tile([NB, NB], F32, tag="d_t")
            nc.scalar.activation(d_t[:], fld("trefd_m"), AF.Exp, scale=-1.0)
            # d_t is ACT-produced while the A-matmul's rhs w_sb is DVE-produced;
            # route d_t through DVE so that matmul needs only one (DVE) wait.
            d_t2 = cp.tile([NB, NB], F32, tag="d_t2")
            nc.vector.tensor_copy(d_t2[:], d_t[:])

            # W transposed: per-block matmuls write free-dim columns (matmul
            # out/lhsT base partition must be 0/32/64, so [32,64] rows won't do)
            psum_wt = ps1.tile([S, NB], F32, tag="pwt")
            for b in range(NB):
                nc.tensor.matmul(
                    psum_wt[:, b:b + 1], fld("onehot_t")[:, b * S:(b + 1) * S],
                    v_t[:, b:b + 1], start=True, stop=True)
            wt_sb = cp.tile([S, NB], F32, tag="wt_sb")
            nc.vector.tensor_copy(wt_sb[:], psum_wt[:])
            # true transpose [64,32]->[32,64] on the PE (DVE stream transpose
            # does not lower in this walrus build)
            psum_w2 = ps1.tile([NB, S], F32, tag="pa", name="psum_w2")
            nc.tensor.transpose(psum_w2[:], wt_sb[:], fld("ident64"))
            w_sb = cp.tile([NB, S], F32, tag="w_sb")
            nc.vector.tensor_copy(w_sb[:], psum_w2[:])

            psum_a = ps1.tile([NB, S], F32, tag="pa")
            nc.tensor.matmul(psum_a[:], d_t2[:], w_sb[:], start=True, stop=True)
            a_sb = cp.tile([NB, S], F32, tag="a_sb")
            nc.vector.tensor_copy(a_sb[:], psum_a[:])

            # A rows for my 4 blocks, flattened to one [1, 256] row so later
            # matmul rhs slices sit at base partition 0.
            amy_flat = cp.tile([1, NBLK * S], F32, tag="amy_flat")
            for k in range(NBLK):
                psum_amy = ps1.tile([1, S], F32, tag="pamy")
                nc.tensor.matmul(psum_amy[:], fld("sel")[:, k:k + 1], a_sb[:],
                                 start=True, stop=True)
                nc.vector.tensor_copy(amy_flat[0:1, k * S:(k + 1) * S], psum_amy[:])

            # ---- term1: per-row-block intensities, relu+log ----
            logacc = cp.tile([BS, NBLK], F32, tag="logacc")
            for k in range(NBLK):
                vmask = wp.tile([BS, BS], F32, tag="vmask")
                nc.vector.tensor_scalar_mul(vmask[:], fld("lmask"), vmy_t[:, k:k + 1])
                psum_r = ps1.tile([BS, S], F32, tag="pr", bufs=2)
                nc.tensor.matmul(psum_r[:], ones_row[:],
                                 amy_flat[0:1, k * S:(k + 1) * S],
                                 start=True, stop=False)
                nc.tensor.matmul(psum_r[:], vmask[:],
                                 fld("onehot_my")[:, k * S:(k + 1) * S],
                                 start=False, stop=True)
                vp = wp.tile([BS, S], F32, tag="vp")
                nc.vector.tensor_scalar_mul(vp[:], psum_r[:], umy_t[:, k:k + 1])
                junk = wp.tile([BS, S], F32, tag="junk")
                red = wp.tile([BS, 1], F32, tag="red")
                nc.vector.tensor_tensor(junk[:], vp[:],
                                        fld("alphag_my")[:, k * S:(k + 1) * S],
                                        ALU.mult)
                nc.vector.reduce_sum(red[:], junk[:], mybir.AxisListType.X)
                lam = wp.tile([BS, 1], F32, tag="lam")
                nc.scalar.activation(lam[:], red[:], AF.Relu,
                                     bias=fld("mug_my")[:, k:k + 1])
                nc.scalar.activation(logacc[:, k:k + 1], lam[:], AF.Ln)

            t1red = cp.tile([BS, 1], F32, tag="t1red")
            nc.vector.reduce_sum(t1red[:], logacc[:], mybir.AxisListType.X)
            psum_s1 = ps1.tile([1, 1], F32, tag="pamy", name="psum_s1")
            nc.tensor.matmul(psum_s1[:], t1red[:], ones_col[:], start=True, stop=True)

            # ---- term2: dense masked exp + accumulating matmuls ----
            dt2 = cp.tile([BS, W2], F32, tag="dt2")
            nc.vector.tensor_tensor(dt2[:], fld("tg_t2"), fld("times_t2"),
                                    ALU.subtract)
            mbig = cp.tile([BS, W2], F32, tag="mbig")
            nc.vector.tensor_scalar(mbig[:], dt2[:], 0.0, BIG, ALU.is_le, ALU.mult)
            dtm = cp.tile([BS, W2], F32, tag="dtm")
            nc.vector.tensor_tensor(dtm[:], dt2[:], mbig[:], ALU.add)
            e2 = cp.tile([BS, W2], F32, tag="e2")
            nc.scalar.activation(e2[:], dtm[:], AF.Exp, scale=-1.0)

            psum_ct = ps2.tile([S, GS], F32, tag="pct")
            for b in range(NB):
                nc.tensor.matmul(psum_ct[:], fld("onehot_t")[:, b * S:(b + 1) * S],
                                 e2[:, b * GS:(b + 1) * GS],
                                 start=(b == 0), stop=(b == NB - 1))
            ct_sb = cp.tile([S, GS], F32, tag="ct_sb")
            nc.vector.tensor_copy(ct_sb[:], psum_ct[:])
            psum_v2 = ps2.tile([S, GS], F32, tag="pv2")
            nc.tensor.matmul(psum_v2[:], fld("alpha_t"), ct_sb[:],
                             start=True, stop=True)
            r2 = cp.tile([S, GS], F32, tag="r2")
            nc.scalar.activation(r2[:], psum_v2[:], AF.Relu, bias=fld("mu_col"))
            junk2 = cp.tile([S, GS], F32, tag="junk2")
            t2red = cp.tile([S, 1], F32, tag="t2red")
            nc.vector.tensor_tensor(junk2[:], r2[:], fld("gmask"), ALU.mult)
            nc.vector.reduce_sum(t2red[:], junk2[:], mybir.AxisListType.X)
            psum_s2 = ps1.tile([1, 1], F32, tag="pamy", name="psum_s2")
            nc.tensor.matmul(psum_s2[:], t2red[:], ones_col[0:S, :],
                             start=True, stop=True)

            out_sb = cp.tile([1, 2], F32, tag="out_sb")
            nc.vector.tensor_copy(out_sb[0:1, 0:1], psum_s1[:])
            nc.vector.tensor_copy(out_sb[0:1, 1:2], psum_s2[:])
            nc.sync.dma_start(outd[:], out_sb[:])

    return nc


def _host_inputs(data, mu_param, alpha, beta):
    times = np.ascontiguousarray(data[:, 0], dtype=np.float32)
    sp = data[:, 1].astype(np.int32)
    beta = float(beta)
    mu = np.asarray(mu_param, dtype=np.float32)
    alpha_f = np.asarray(alpha, dtype=np.float32) * np.float32(beta)

    tref = times[::BS]                                       # [32]
    tdel_all = ((times.reshape(NB, BS).T - tref[None, :]) * beta).astype(np.float32)
    onehot = np.zeros((N, S), np.float32)
    onehot[np.arange(N), sp] = 1.0
    oh3 = onehot.reshape(NB, BS, S).transpose(1, 0, 2)       # [j, b, s']
    onehot_t = oh3.reshape(BS, NB * S)
    trefd = (beta * (tref[None, :] - tref[:, None])).astype(np.float32)
    bidx = np.arange(NB)
    trefd_m = np.where(bidx[:, None] < bidx[None, :], trefd,
                       np.float32(BIG)).astype(np.float32)
    lmask = (np.arange(BS)[:, None] < np.arange(BS)[None, :]).astype(np.float32)
    times_t2 = np.repeat(times.reshape(NB, BS).T, GS, axis=1)  # [128, 416]
    alpha_t = alpha_f.T
    mu_col = mu[:, None]
    ident64 = np.eye(S, dtype=np.float32)
    tgrid = np.linspace(T0, T1, INT_RES).astype(np.float32)

    in_maps = []
    for c in range(8):
        myb = [c, 31 - c, 8 + c, 23 - c]
        sel = np.zeros((NB, NBLK), np.float32)
        for k, b in enumerate(myb):
            sel[b, k] = 1.0
        tdel_my = tdel_all[:, myb]
        onehot_my = oh3[:, myb, :].reshape(BS, NBLK * S)
        rows = np.concatenate([sp[b * BS:(b + 1) * BS] for b in myb])
        alphag_my = alpha_f[rows].reshape(NBLK, BS, S).transpose(1, 0, 2) \
                                .reshape(BS, NBLK * S)
        mug_my = mu[rows].reshape(NBLK, BS).T
        gcols = np.zeros(GS, np.float32)
        nvalid = max(0, min(GS, G - c * GS))
        gcols[:nvalid] = tgrid[c * GS:c * GS + nvalid]
        tg_t2 = np.broadcast_to(np.tile(gcols, NB), (BS, W2))
        grow = np.zeros(GS, np.float32)
        grow[:nvalid] = np.float32(T1 / INT_RES)
        gmask = np.broadcast_to(grow, (S, GS))

        vals = {
            "tdel_all": tdel_all, "tdel_my": tdel_my, "trefd_m": trefd_m,
            "sel": sel, "onehot_t": onehot_t, "onehot_my": onehot_my,
            "alphag_my": alphag_my, "mug_my": mug_my, "lmask": lmask,
            "tg_t2": tg_t2, "times_t2": times_t2, "alpha_t": alpha_t,
            "mu_col": mu_col, "ident64": ident64, "gmask": gmask,
        }
        blob = np.zeros((BS, BLOB_COLS), np.float32)
        for nm, r, cc in _FIELDS:
            blob[0:r, _OFF[nm]:_OFF[nm] + cc] = vals[nm]
        in_maps.append({"blob": blob})
    return in_maps


def kernel(data, mu_param, alpha, beta, _trace=False):
    if "nc" not in _CACHE:
        _CACHE["nc"] = _build_program()
    nc = _CACHE["nc"]
    in_maps = _host_inputs(np.asarray(data), mu_param, alpha, beta)
    res = run_bass_kernel_spmd(nc, in_maps, list(range(8)), trace=_trace)
    t1 = sum(float(r["outd"][0, 0]) for r in res.results)
    t2 = sum(float(r["outd"][0, 1]) for r in res.results)
    out = np.float32(t1 - t2)
    if _trace:
        return np.asarray(out), res
    return np.asarray(out)



# revision 12
# speedup vs baseline: 1.8054x; 1.8054x over previous
"""Multivariate Hawkes log-likelihood on 8 Trainium2 NeuronCores (v2).

Math: per 128-event block b with tref_b = times[b*128],
    v[j,b]   = exp((t_jb - tref_b)*beta)
    e2[j,b,g]= exp(-(t_g - t_jb)*beta) * [t_jb < t_g]
One fused PE stream computes, per block, [W_col | ct_cols]:
    psum_f[s', b*14+q] = sum_j onehot[j,b,s'] * packed[j, b*14+q]
where packed interleaves v (q=0) and e2 (q=1..13). A log2 fold-tree sums the
32 blocks into ct[s',g]; the W columns (stride-14 view) feed term1's
cross-block path via Q[i,b'] = sum_s' alphagu[i,s']*W[b',s'] (one matmul per
row block) and a fused DVE multiply-reduce against the decay matrix
dmyB[i, k*32+b'] = exp(-(tref_bk - tref_b')*beta)[b'<bk], chained with the
in-block strict-lower matmul reduce. All matmul operands are bf16 (single-pass
PE, fast weight load); exp args and accumulations stay fp32.

Sharding: row blocks {c, 31-c, 8+c, 23-c} per core, time-grid columns
[c*13, c*13+13). Host does O(N) layout prep (one-hots, masks, alpha row
gathers with the per-event u=exp(-(t-tref)) factor folded in) plus the final
16-scalar gather; the mu part of term2 is a host constant.

The walrus build rejects instructions with >1 embedded sync wait: inputs ride
in THREE DMA blobs (f32 early fields / onehot_t bf16 / rest bf16), each engine
touches each blob it reads once to absorb that DMA semaphore, and the
kernel-tail drain is split per-semaphore (monkeypatch below).
"""

import numpy as np
import ml_dtypes

import concourse.bass as bass
import concourse.mybir as mybir
import concourse.tile as tile
from concourse.bass_utils import run_bass_kernel_spmd
from concourse.vector_clock import ScopedClock, VectorClock


def _split_drain_and_barrier(self, tick_clock, wait_clock):
    gclock = tick_clock.global_clock
    for proc in range(len(gclock)):
        tick = gclock[proc]
        if tick <= 0:
            continue
        vc1 = VectorClock()
        vc1.require_at_least(proc, tick)
        di = self.nc.sync.drain()
        wait_clock.add_sem_waits(di.ins, ScopedClock({None: vc1}))
    self.nc.all_engine_barrier()
    assert self.sems is not None
    popped = self.nc._tile_sem_poison_stack.pop()
    assert popped is self._sem_poison
    self.nc.clear_and_free_semaphores(list(self.sems.allocated().values()))
    self.nc.all_engine_barrier()


tile.TileContext._drain_and_barrier = _split_drain_and_barrier

N, NB, BS, S, G, GS = 4096, 32, 128, 64, 100, 13
T0, T1, INT_RES = 0.0, 100.0, 100
NBLK = 4
QW = 14                   # packed cols per block: [v | 13 e2]
W2 = NB * GS              # 416
BIG = 1.0e5
F32 = mybir.dt.float32
BF16 = mybir.dt.bfloat16
AF = mybir.ActivationFunctionType
ALU = mybir.AluOpType

# f32 blob fields: name -> cols (rows always 128 unless noted)
_F32 = [("earg_v", NB), ("tdel_my", NBLK), ("trefd_bc", NB * NBLK),
        ("tg_t2", W2), ("times_t2", W2), ("mug_my", NBLK), ("gmask", GS)]
_O32, _c = {}, 0
for _n, _w in _F32:
    _O32[_n] = _c
    _c += _w
C32 = _c

# bf16 blob B2 fields
_B2 = [("onehot_my", NBLK * S), ("alphagu", NBLK * S), ("alphaguT", NBLK * BS),
       ("alpha_t", S), ("lmask", BS)]
_OB2, _c = {}, 0
for _n, _w in _B2:
    _OB2[_n] = _c
    _c += _w
CB2 = _c

_CACHE = {}


def _build_program():
    nc = bass.Bass()
    blob32 = nc.dram_tensor("blob32", [BS, C32], F32, kind="ExternalInput")
    blob_oh = nc.dram_tensor("blob_oh", [BS, NB * S], BF16, kind="ExternalInput")
    blob_b2 = nc.dram_tensor("blob_b2", [BS, CB2], BF16, kind="ExternalInput")
    outd = nc.dram_tensor("outd", [1, 2], F32, kind="ExternalOutput")

    with tile.TileContext(nc) as tc:
        with (
            tc.tile_pool(name="const", bufs=1) as cp,
            tc.tile_pool(name="ps1", bufs=1, space=bass.MemorySpace.PSUM) as ps1,
            tc.tile_pool(name="ps2", bufs=1, space=bass.MemorySpace.PSUM) as ps2,
        ):
            b32 = cp.tile([BS, C32], F32, tag="b32")
            oh = cp.tile([BS, NB * S], BF16, tag="oh")
            b2 = cp.tile([BS, CB2], BF16, tag="b2")
            nc.sync.dma_start(b32[:], blob32[:])
            nc.sync.dma_start(oh[:], blob_oh[:])
            nc.sync.dma_start(b2[:], blob_b2[:])

            def f32f(name, rows=BS):
                return b32[0:rows, _O32[name]:_O32[name] + dict(_F32)[name]]

            def b2f(name, rows=BS):
                return b2[0:rows, _OB2[name]:_OB2[name] + dict(_B2)[name]]

            ones_col = cp.tile([BS, 1], F32, tag="ones_col")
            nc.vector.memset(ones_col[:], 1.0)

            # touches: absorb each blob's DMA semaphore per consuming engine
            acttch = cp.tile([1, 1], F32, tag="acttch")
            nc.scalar.copy(acttch[:], b32[0:1, 0:1])
            dvetch = cp.tile([1, 1], F32, tag="dvetch")
            nc.vector.tensor_copy(dvetch[:], b32[0:1, 0:1])

            # ---- ACT: exps (order matters: dmyB before vmy; e2 after DVE dtm)
            packed = cp.tile([BS, NB * QW], BF16, tag="packed")
            p3 = packed[:].rearrange("p (b q) -> p b q", q=QW)
            nc.scalar.activation(p3[:, :, 0:1],
                                 f32f("earg_v").rearrange("p (b o) -> p b o", o=1),
                                 AF.Exp, scale=-1.0)
            dmyB = cp.tile([BS, NB * NBLK], F32, tag="dmyB")
            nc.scalar.activation(dmyB[:], f32f("trefd_bc"), AF.Exp, scale=-1.0)
            vmy = cp.tile([BS, NBLK], F32, tag="vmy")
            nc.scalar.activation(vmy[:], f32f("tdel_my"), AF.Exp, scale=-1.0)

            # ---- DVE: masked exp arg for e2
            dt2 = cp.tile([BS, W2], F32, tag="dt2")
            nc.vector.tensor_tensor(dt2[:], f32f("tg_t2"), f32f("times_t2"),
                                    ALU.subtract)
            mbig = cp.tile([BS, W2], F32, tag="mbig")
            nc.vector.tensor_scalar(mbig[:], dt2[:], 0.0, BIG, ALU.is_le, ALU.mult)
            dtm = cp.tile([BS, W2], F32, tag="dtm")
            nc.vector.tensor_tensor(dtm[:], dt2[:], mbig[:], ALU.add)
            nc.scalar.activation(p3[:, :, 1:QW],
                                 dtm[:].rearrange("p (b g) -> p b g", g=GS),
                                 AF.Exp, scale=-1.0)

            # ---- PE: touch onehot blob, then 32 fused [W|ct] matmuls
            ptch1 = ps1.tile([1, 1], F32, tag="ptch", name="ptch1")
            nc.tensor.matmul(ptch1[:], oh[0:1, 0:1], oh[0:1, 0:1],
                             start=True, stop=True)
            psum_f = ps1.tile([S, NB * QW], F32, tag="pf")
            for b in range(NB):
                nc.tensor.matmul(psum_f[:, b * QW:(b + 1) * QW],
                                 oh[:, b * S:(b + 1) * S],
                                 packed[:, b * QW:(b + 1) * QW],
                                 start=True, stop=True)

            # ---- DVE: touch b2, vmasks, fused copy, fold tree
            dvetch2 = cp.tile([1, 1], BF16, tag="dvetch2")
            nc.vector.tensor_copy(dvetch2[:], b2[0:1, 0:1])
            vmasks = []
            for k in range(NBLK):
                vm = cp.tile([BS, BS], BF16, tag=f"vmask{k}")
                nc.vector.tensor_scalar_mul(vm[:], b2f("lmask"), vmy[:, k:k + 1])
                vmasks.append(vm)
            fused = cp.tile([S, NB * QW], BF16, tag="fused")
            nc.vector.tensor_copy(fused[:], psum_f[:])
            w = NB * QW // 2
            prev = fused
            folds = []
            while w >= QW * 2:
                nxt = cp.tile([S, w], F32, tag=f"fold{w}")
                nc.vector.tensor_tensor(nxt[:], prev[0:S, 0:w],
                                        prev[0:S, w:2 * w], ALU.add)
                folds.append(nxt)
                prev = nxt
                w //= 2
            ct14 = cp.tile([S, QW], BF16, tag="ct14")
            nc.vector.tensor_tensor(ct14[:], prev[0:S, 0:QW],
                                    prev[0:S, QW:2 * QW], ALU.add)

            # ---- PE: touch b2, Q matmuls, in-block matmuls, term2 alpha matmul
            ptch2 = ps1.tile([1, 1], F32, tag="ptch", name="ptch2")
            nc.tensor.matmul(ptch2[:], b2[0:1, 0:1], b2[0:1, 0:1],
                             start=True, stop=True)
            wt_view = fused[0:S, 0:NB * QW:QW]          # [64, 32] = W^T
            pq = ps2.tile([BS, NB * NBLK], F32, tag="pq")
            pr = ps2.tile([BS, S * NBLK], F32, tag="pr")
            psum_q = [pq[:, k * NB:(k + 1) * NB] for k in range(NBLK)]
            psum_r = [pr[:, k * S:(k + 1) * S] for k in range(NBLK)]
            for k in range(NBLK):
                nc.tensor.matmul(psum_q[k],
                                 b2f("alphaguT", S)[:, k * BS:(k + 1) * BS],
                                 wt_view, start=True, stop=True)
            for k in range(NBLK):
                nc.tensor.matmul(psum_r[k], vmasks[k][:],
                                 b2f("onehot_my")[:, k * S:(k + 1) * S],
                                 start=True, stop=True)
            psum_v2 = ps1.tile([S, GS], F32, tag="pv2")
            nc.tensor.matmul(psum_v2[:], b2f("alpha_t", S), ct14[0:S, 1:QW],
                             start=True, stop=True)

            # ---- DVE: multiply-reduces (cross-block, in-block), lam assembly
            redq = cp.tile([BS, NBLK], F32, tag="redq")
            redr = cp.tile([BS, NBLK], F32, tag="redr")
            for k in range(NBLK):
                jq = cp.tile([BS, NB], F32, tag=f"jq{k}")
                nc.vector.tensor_tensor(jq[:], psum_q[k],
                                        dmyB[:, k * NB:(k + 1) * NB], ALU.mult)
                nc.vector.reduce_sum(redq[:, k:k + 1], jq[:],
                                     mybir.AxisListType.X)
            for k in range(NBLK):
                jr = cp.tile([BS, S], F32, tag=f"jr{k}")
                nc.vector.tensor_tensor(jr[:], psum_r[k],
                                        b2f("alphagu")[:, k * S:(k + 1) * S],
                                        ALU.mult)
                nc.vector.reduce_sum(redr[:, k:k + 1], jr[:],
                                     mybir.AxisListType.X)
            redm = cp.tile([BS, NBLK], F32, tag="redm")
            nc.vector.tensor_tensor(redm[:], redq[:], f32f("mug_my"), ALU.add)
            lamm = cp.tile([BS, NBLK], F32, tag="lamm")
            nc.vector.tensor_tensor(lamm[:], redm[:], redr[:], ALU.add)

            # ---- ACT: log + row-sum; PE: partition sums; out
            lnout = cp.tile([BS, NBLK], F32, tag="lnout")
            t1red = cp.tile([BS, 1], F32, tag="t1red")
            nc.scalar.activation(lnout[:], lamm[:], AF.Ln, accum_out=t1red[:])
            pss = ps1.tile([1, 2], F32, tag="pss")
            nc.tensor.matmul(pss[0:1, 0:1], t1red[:], ones_col[:],
                             start=True, stop=True)
            junk2 = cp.tile([S, GS], F32, tag="junk2")
            t2red = cp.tile([S, 1], F32, tag="t2red")
            nc.vector.tensor_tensor(junk2[:], psum_v2[:], f32f("gmask", S),
                                    ALU.mult)
            nc.vector.reduce_sum(t2red[:], junk2[:], mybir.AxisListType.X)
            nc.tensor.matmul(pss[0:1, 1:2], t2red[:], ones_col[0:S, :],
                             start=True, stop=True)

            out_sb = cp.tile([1, 2], F32, tag="out_sb")
            nc.vector.tensor_copy(out_sb[:], pss[:])
            nc.sync.dma_start(outd[:], out_sb[:])

    return nc


def _host_inputs(data, mu_param, alpha, beta):
    times = np.ascontiguousarray(data[:, 0], dtype=np.float64)
    sp = data[:, 1].astype(np.int32)
    beta = float(beta)
    mu = np.asarray(mu_param, dtype=np.float64)
    alpha_f = np.asarray(alpha, dtype=np.float64) * beta

    tref = times[::BS]                       # [32]
    tj = times.reshape(NB, BS).T             # [128, 32]
    spj = sp.reshape(NB, BS).T               # [128, 32]
    tgrid = np.linspace(T0, T1, INT_RES)

    earg_v = (tref[None, :] - tj) * beta                      # [128, 32]
    times_t2 = np.repeat(tj, GS, axis=1)                      # [128, 416]
    lmask = (np.arange(BS)[:, None] < np.arange(BS)[None, :]).astype(np.float32)
    onehot = np.zeros((BS, NB, S), np.float32)
    onehot[np.arange(BS)[:, None], np.arange(NB)[None, :], spj] = 1.0
    oh_flat = onehot.reshape(BS, NB * S)
    bidx = np.arange(NB)

    in_maps = []
    for c in range(8):
        myb = [c, 31 - c, 8 + c, 23 - c]
        tdel_my = (tref[myb][None, :] - tj[:, myb]) * beta    # [128, 4]
        trefd = np.concatenate([
            np.where(bidx < b, (tref[b] - tref) * beta, BIG) for b in myb])
        trefd_bc = np.broadcast_to(trefd, (BS, NB * NBLK))    # [128, 128]
        gcols = np.zeros(GS)
        nvalid = max(0, min(GS, G - c * GS))
        gcols[:nvalid] = tgrid[c * GS:c * GS + nvalid]
        tg_t2 = np.broadcast_to(np.tile(gcols, NB), (BS, W2)) * beta
        mug_my = mu[spj[:, myb]]                              # [128, 4]
        gm = np.zeros((BS, GS), np.float32)
        gm[0:S, :nvalid] = T1 / INT_RES

        u = np.exp(-(tj[:, myb] - tref[myb][None, :]) * beta)  # [128, 4]
        alphagu = alpha_f[spj[:, myb]] * u[:, :, None]        # [128, 4, 64]
        alphagu_f = alphagu.reshape(BS, NBLK * S)
        # alphaguT[s', k*128+i] = alphagu[i, k, s']
        alphaguT = np.zeros((BS, NBLK * BS), np.float64)
        alphaguT[0:S, :] = alphagu.transpose(2, 1, 0).reshape(S, NBLK * BS)
        onehot_my = onehot[:, myb, :].reshape(BS, NBLK * S)
        alpha_tt = np.zeros((BS, S))
        alpha_tt[0:S, :] = alpha_f.T

        b32 = np.zeros((BS, C32), np.float32)
        for nm, wdt in _F32:
            val = {"earg_v": earg_v, "tdel_my": tdel_my, "trefd_bc": trefd_bc,
                   "tg_t2": times_t2 * 0 + tg_t2, "times_t2": times_t2 * beta,
                   "mug_my": mug_my, "gmask": gm}[nm]
            b32[0:val.shape[0], _O32[nm]:_O32[nm] + wdt] = val
        bb2 = np.zeros((BS, CB2), np.float32)
        for nm, wdt in _B2:
            val = {"onehot_my": onehot_my, "alphagu": alphagu_f,
                   "alphaguT": alphaguT, "alpha_t": alpha_tt,
                   "lmask": lmask}[nm]
            bb2[0:val.shape[0], _OB2[nm]:_OB2[nm] + wdt] = val
        in_maps.append({
            "blob32": b32,
            "blob_oh": oh_flat.astype(ml_dtypes.bfloat16),
            "blob_b2": bb2.astype(ml_dtypes.bfloat16),
        })
    return in_maps


def kernel(data, mu_param, alpha, beta, _trace=False):
    if "nc" not in _CACHE:
        _CACHE["nc"] = _build_program()
    nc = _CACHE["nc"]
    in_maps = _host_inputs(np.asarray(data), mu_param, alpha, beta)
    res = run_bass_kernel_spmd(nc, in_maps, list(range(8)), trace=_trace)
    t1 = sum(float(r["outd"][0, 0]) for r in res.results)
    t2 = sum(float(r["outd"][0, 1]) for r in res.results)
    t2 += float(np.sum(np.asarray(mu_param, np.float64))) * G * (T1 / INT_RES)
    out = np.float32(t1 - t2)
    if _trace:
        return np.asarray(out), res
    return np.asarray(out)
